# revision 1
# baseline (speedup 1.0000x reference)
"""Trainium2 Bass kernel for nn_ProjectionLayer: mean-pool + projection +
L2-normalize + cosine-sim matrix / pairwise-distance denominator.

Reference math (fp32):
    g = mean(features, axis=2) @ W.T + bias        # [b, out_c]
    g = g / max(||g||_row, 1e-12)                  # L2 normalize rows
    sim = g @ g.T                                  # [b, b]
    dist = ||g + 1e-6||_row                        # [b]
    out = sim / max(dist_i, dist_j, 1e-8)

Sharding: data-parallel over batch (64 rows per core, 8 cores); the
normalized features are AllGather'd (128 KB/rank) so every core can form its
[64, 512] block of the output.
"""

import sys

if "/opt/trn_rl_repo" not in sys.path:
    sys.path.insert(0, "/opt/trn_rl_repo")

import numpy as np

# Problem shapes (hardcoded per contract)
B_FULL = 512     # batch
C_IN = 2048      # in channels (contraction dim of projection)
T_POOL = 196     # pooled (time) dim
O_OUT = 512      # out channels
N_CORES = 8

PD_EPS = 1e-6
NORM_EPS = 1e-12
DENO_EPS = 1e-8


def build_kernel(b_full, c_in, t_pool, o_out, n_cores, bg=4, cpp=4):
    """Emit the Bass module (SPMD program, identical on every core).

    cpp = channels per partition in the feature-pooling layout: partition p of
    chunk k holds channels c = 512k + cpp*p + j (j in [0, cpp)), which makes
    each DMA descriptor a cpp*t_pool*4-byte contiguous run (fewer, bigger
    descriptors -> HWDGE keeps the 16 SDMA engines fed). The channel
    interleave is undone for free by building W^T chunks from stride-cpp
    column slices of W.
    """
    import concourse.mybir as mybir
    import concourse.tile as tile
    from concourse import bacc
    from concourse.masks import make_identity

    f32 = mybir.dt.float32
    AL = mybir.AluOpType

    bc = b_full // n_cores          # batch rows per core
    span = 128 * cpp                # channels per chunk
    nk = c_in // span               # chunks
    nbg = bc // bg                  # feature-tile batch groups
    oc = o_out // 128               # out-channel chunks
    qb = b_full // 128              # gathered-row chunks
    assert bc % bg == 0 and c_in % span == 0 and o_out % 128 == 0
    assert b_full % 128 == 0 and o_out <= 512 and b_full <= 512

    nc = bacc.Bacc("TRN2", target_bir_lowering=False, debug=False,
                   enable_asserts=False, num_devices=n_cores)
    feat = nc.dram_tensor("features", [bc, c_in, t_pool], f32,
                          kind="ExternalInput").ap()
    w_in = nc.dram_tensor("w", [o_out, c_in], f32, kind="ExternalInput").ap()
    bias_in = nc.dram_tensor("bias", [1, o_out], f32, kind="ExternalInput").ap()
    out_d = nc.dram_tensor("out", [bc, b_full], f32, kind="ExternalOutput").ap()

    with tile.TileContext(nc) as tc:
        with (
            tc.tile_pool(name="const", bufs=1) as constp,
            tc.tile_pool(name="wload", bufs=1) as wlp,
            tc.tile_pool(name="wtp", bufs=1) as wtp,
            tc.tile_pool(name="featp", bufs=4) as fp,
            tc.tile_pool(name="lhsp", bufs=1) as lp,
            tc.tile_pool(name="postp", bufs=1) as pp,
            tc.tile_pool(name="psrot", bufs=2, space="PSUM") as psp,
            tc.tile_pool(name="psfix", bufs=1, space="PSUM") as psgp,
            tc.tile_pool(name="dram", bufs=1, space="DRAM") as dp,
        ):
            # ---- constants ----
            ident = constp.tile([128, 128], f32, name="ident")
            make_identity(nc, ident)
            ones = constp.tile([1, bc], f32, name="ones")
            nc.vector.memset(ones, 1.0)
            bias_sb = constp.tile([1, o_out], f32, name="bias_sb")
            nc.sync.dma_start(bias_sb[:], bias_in[:])

            # ---- W^T / t_pool, interleave-matched layout ----
            # wt4[k][j] rows: partition p <-> channel c = span*k + cpp*p + j
            wl = []
            for l in range(oc):
                wli = wlp.tile([128, c_in], f32, name=f"wl{l}")
                nc.sync.dma_start(wli[:], w_in[l * 128:(l + 1) * 128, :])
                wl.append(wli)
            wt4 = []
            for k in range(nk):
                for j in range(cpp):
                    pswt = psp.tile([128, o_out], f32, name="pswt", tag="rot")
                    for l in range(oc):
                        src = wl[l][:, k * span:(k + 1) * span].rearrange(
                            "o (p j) -> o p j", j=cpp)[:, :, j]
                        nc.tensor.transpose(pswt[:, l * 128:(l + 1) * 128],
                                            src, ident[:])
                    wtk = wtp.tile([128, o_out], f32, name=f"wt{k}_{j}")
                    nc.scalar.mul(wtk[:], pswt[:], 1.0 / t_pool)
                    wt4.append(wtk)

            # ---- pooling: p4[k][128p, bc b, cpp j] = sum_t features ----
            p4 = [lp.tile([128, bc, cpp], f32, name=f"p4_{k}") for k in range(nk)]
            idma = 0
            for ibg in range(nbg):
                for k in range(nk):
                    ft = fp.tile([128, bg, cpp, t_pool], f32, name="ft")
                    src = feat[ibg * bg:(ibg + 1) * bg,
                               k * span:(k + 1) * span, :].rearrange(
                                   "b (p j) t -> p b j t", j=cpp)
                    # alternate the two HWDGE rings (SP / ACT) so descriptor
                    # generation is not serialized on one engine
                    dma_eng = nc.sync if idma % 2 == 0 else nc.scalar
                    dma_eng.dma_start(ft[:], src)
                    nc.vector.reduce_sum(p4[k][:, ibg * bg:(ibg + 1) * bg, :],
                                         ft[:], axis=mybir.AxisListType.X)
                    idma += 1

            # ---- projection: g = pooled/t @ W.T + bias  -> PSUM [bc, o_out] ----
            gps = psgp.tile([bc, o_out], f32, name="gps")
            for k in range(nk):
                for j in range(cpp):
                    nc.tensor.matmul(gps[:], p4[k][:, :, j], wt4[k * cpp + j][:],
                                     start=(k == 0 and j == 0), stop=False)
            nc.tensor.matmul(gps[:], ones[:], bias_sb[:], start=False, stop=True)

            # ---- L2 normalize rows ----
            gsb = pp.tile([bc, o_out], f32, name="gsb")
            nc.scalar.copy(gsb[:], gps[:])
            scr = pp.tile([bc, o_out], f32, name="scr")
            nrm2 = pp.tile([bc, 1], f32, name="nrm2")
            nc.vector.tensor_mul(scr[:], gsb[:], gsb[:])
            nc.vector.reduce_sum(nrm2[:], scr[:], axis=mybir.AxisListType.X)
            nrm = pp.tile([bc, 1], f32, name="nrm")
            nc.scalar.sqrt(nrm[:], nrm2[:])
            nmax = pp.tile([bc, 1], f32, name="nmax")
            nc.vector.tensor_scalar_max(nmax[:], nrm[:], NORM_EPS)
            rinv = pp.tile([bc, 1], f32, name="rinv")
            nc.vector.reciprocal(rinv[:], nmax[:])
            gn = pp.tile([bc, o_out], f32, name="gn")
            nc.scalar.mul(gn[:], gsb[:], rinv[:])

            # local dist column: ||gn + eps||_row  [bc, 1]
            nc.vector.tensor_scalar_add(scr[:], gn[:], PD_EPS)
            nc.vector.tensor_mul(scr[:], scr[:], scr[:])
            dl2 = pp.tile([bc, 1], f32, name="dl2")
            nc.vector.reduce_sum(dl2[:], scr[:], axis=mybir.AxisListType.X)
            dl = pp.tile([bc, 1], f32, name="dl")
            nc.scalar.sqrt(dl[:], dl2[:])

            # ---- AllGather normalized features ----
            ag_in = dp.tile([bc, o_out], f32, name="ag_in")
            ag_out = dp.tile([b_full, o_out], f32, name="ag_out",
                             addr_space="Shared")
            nc.sync.dma_start(ag_in[:], gn[:])
            nc.gpsimd.collective_compute(
                "AllGather", AL.bypass,
                replica_groups=[list(range(n_cores))],
                ins=[ag_in.opt()], outs=[ag_out.opt()],
            )

            gf = []
            for q in range(qb):
                gfq = pp.tile([128, o_out], f32, name=f"gf{q}")
                nc.sync.dma_start(gfq[:], ag_out[q * 128:(q + 1) * 128, :])
                gf.append(gfq)

            # dist for all gathered rows: [128, qb]
            scrq = pp.tile([128, o_out], f32, name="scrq")
            d2 = pp.tile([128, qb], f32, name="d2")
            for q in range(qb):
                nc.vector.tensor_scalar_add(scrq[:], gf[q][:], PD_EPS)
                nc.vector.tensor_mul(scrq[:], scrq[:], scrq[:])
                nc.vector.reduce_sum(d2[:, q:q + 1], scrq[:],
                                     axis=mybir.AxisListType.X)
            dist = pp.tile([128, qb], f32, name="dist")
            nc.scalar.sqrt(dist[:], d2[:])

            # dist as a row vector [1, b_full] (PE transpose of columns)
            psdr = psp.tile([1, b_full], f32, name="psdr", tag="rot")
            for q in range(qb):
                nc.tensor.transpose(psdr[:, q * 128:(q + 1) * 128],
                                    dist[:, q:q + 1], ident[:])
            distrow = pp.tile([1, b_full], f32, name="distrow")
            nc.scalar.copy(distrow[:], psdr[:])

            # gathered features transposed: gt[m][128 o, b_full]
            gt = []
            for m in range(oc):
                psgt = psp.tile([128, b_full], f32, name="psgt", tag="rot")
                for q in range(qb):
                    nc.tensor.transpose(psgt[:, q * 128:(q + 1) * 128],
                                        gf[q][:, m * 128:(m + 1) * 128],
                                        ident[:])
                gtm = pp.tile([128, b_full], f32, name=f"gt{m}")
                nc.vector.tensor_copy(gtm[:], psgt[:])
                gt.append(gtm)

            # local rows transposed: gl[m][128 o, bc]
            gl = []
            for m in range(oc):
                psgl = psp.tile([128, bc], f32, name="psgl", tag="rot")
                nc.tensor.transpose(psgl[:], gn[:, m * 128:(m + 1) * 128],
                                    ident[:bc, :bc])
                glm = pp.tile([128, bc], f32, name=f"gl{m}")
                nc.vector.tensor_copy(glm[:], psgl[:])
                gl.append(glm)

            # sim block: [bc, b_full] = gn @ gf.T
            sps = psgp.tile([bc, b_full], f32, name="sps")
            for m in range(oc):
                nc.tensor.matmul(sps[:], gl[m][:], gt[m][:],
                                 start=(m == 0), stop=(m == oc - 1))

            # deno = max(dist_i, dist_j, eps); out = sim / deno
            dps = psgp.tile([bc, b_full], f32, name="dps")
            nc.tensor.matmul(dps[:], ones[:], distrow[:], start=True, stop=True)
            den = pp.tile([bc, b_full], f32, name="den")
            nc.vector.tensor_scalar(den[:], dps[:], dl[:], DENO_EPS,
                                    op0=AL.max, op1=AL.max)
            rden = pp.tile([bc, b_full], f32, name="rden")
            nc.vector.reciprocal(rden[:], den[:])
            outsb = pp.tile([bc, b_full], f32, name="outsb")
            nc.vector.tensor_mul(outsb[:], sps[:], rden[:])
            nc.sync.dma_start(out_d[:], outsb[:])

    nc.compile()
    return nc


_NC_CACHE = {}


def _get_nc():
    key = (B_FULL, C_IN, T_POOL, O_OUT, N_CORES)
    if key not in _NC_CACHE:
        _NC_CACHE[key] = build_kernel(*key)
    return _NC_CACHE[key]


def _run(features, W, bias, trace=False):
    from concourse.bass_utils import run_bass_kernel_spmd

    feats = np.ascontiguousarray(np.asarray(features, dtype=np.float32))
    w_np = np.ascontiguousarray(np.asarray(W, dtype=np.float32))
    bias_np = np.ascontiguousarray(
        np.asarray(bias, dtype=np.float32).reshape(1, O_OUT))
    bc = B_FULL // N_CORES

    nc = _get_nc()
    in_maps = [
        {"features": feats[r * bc:(r + 1) * bc], "w": w_np, "bias": bias_np}
        for r in range(N_CORES)
    ]
    res = run_bass_kernel_spmd(nc, in_maps, core_ids=list(range(N_CORES)),
                               trace=trace)
    out = np.concatenate([res.results[r]["out"] for r in range(N_CORES)], axis=0)
    return out, res.exec_time_ns


def kernel(features, W, bias):
    out, _ = _run(features, W, bias)
    return out



# revision 11
# speedup vs baseline: 1.1028x; 1.1028x over previous
"""Trainium2 Bass kernel for nn_ProjectionLayer: mean-pool + projection +
L2-normalize + cosine-sim matrix / pairwise-distance denominator.

Reference math (fp32):
    g = mean(features, axis=2) @ W.T + bias        # [b, out_c]
    g = g / max(||g||_row, 1e-12)                  # L2 normalize rows
    sim = g @ g.T                                  # [b, b]
    dist = ||g + 1e-6||_row                        # [b]
    out = sim / max(dist_i, dist_j, 1e-8)

Sharding: data-parallel over batch (64 rows per core, 8 cores). The 64 local
rows are processed in 4 chunks of 16: each chunk's pooled features are
projected + normalized and AllGather'd (32 KB/rank) as soon as they are
ready, so 3 of the 4 collectives (and the gathered-side transposes / dist
work) hide under the feature stream; only the last chunk's AllGather is on
the critical path.

Feature DMA: one dma_start per batch row = one fully contiguous 1.6 MB DRAM
read, split into 128 descriptors of 12544 B (channel c = 16*p + j lands on
partition p at free offset j). The 12.5 KB descriptors amortize the ~40 ns
per-packet SDMA overhead that capped the 3136 B-descriptor version at
~315 GB/s.
"""

import sys

if "/opt/trn_rl_repo" not in sys.path:
    sys.path.insert(0, "/opt/trn_rl_repo")

import numpy as np

# Problem shapes (hardcoded per contract)
B_FULL = 512     # batch
C_IN = 2048      # in channels (contraction dim of projection)
T_POOL = 196     # pooled (time) dim
O_OUT = 512      # out channels
N_CORES = 8

PD_EPS = 1e-6
NORM_EPS = 1e-12
DENO_EPS = 1e-8


def build_kernel(b_full, c_in, t_pool, o_out, n_cores, feat_bufs=6):
    import concourse.mybir as mybir
    import concourse.tile as tile
    from concourse import bacc
    from concourse.masks import make_identity

    f32 = mybir.dt.float32
    AL = mybir.AluOpType
    AF = mybir.ActivationFunctionType
    AX = mybir.AxisListType

    bc = b_full // n_cores          # batch rows per core (64)
    cpp = 16                        # channels per partition -> c = 16p + j
    oc = o_out // 128               # out-channel 128-blocks (4)
    cr = 16                         # rows per AG chunk
    nch = bc // cr                  # chunks (4); gathered chunk = 8*16 = 128 rows
    assert cpp * 128 == c_in and nch * cr == bc and n_cores * cr == 128

    nc = bacc.Bacc("TRN2", target_bir_lowering=False, debug=False,
                   enable_asserts=False, num_devices=n_cores)
    feat = nc.dram_tensor("features", [bc, c_in, t_pool], f32,
                          kind="ExternalInput").ap()
    w_in = nc.dram_tensor("w", [o_out, c_in], f32, kind="ExternalInput").ap()
    bias_in = nc.dram_tensor("bias", [1, o_out], f32, kind="ExternalInput").ap()
    out_d = nc.dram_tensor("out", [bc, b_full], f32, kind="ExternalOutput").ap()

    with tile.TileContext(nc) as tc:
        with (
            tc.tile_pool(name="const", bufs=1) as constp,
            tc.tile_pool(name="wload", bufs=1) as wlp,
            tc.tile_pool(name="wtp", bufs=1) as wtp,
            tc.tile_pool(name="featp", bufs=feat_bufs) as fp,
            tc.tile_pool(name="lhsp", bufs=1) as lp,
            tc.tile_pool(name="postp", bufs=1) as pp,
            tc.tile_pool(name="scrp", bufs=2) as sp,
            tc.tile_pool(name="psrot", bufs=2, space="PSUM") as psp,
            tc.tile_pool(name="psg", bufs=2, space="PSUM") as psg,
            tc.tile_pool(name="pstr", bufs=2, space="PSUM") as pst,
            tc.tile_pool(name="pssim", bufs=1, space="PSUM") as psm,
            tc.tile_pool(name="dram", bufs=1, space="DRAM") as dp,
        ):
            # ---- constants ----
            ident = constp.tile([128, 128], f32, name="ident")
            make_identity(nc, ident)
            ones1 = constp.tile([1, bc], f32, name="ones1")
            nc.vector.memset(ones1, 1.0)
            bias_sb = constp.tile([1, o_out], f32, name="bias_sb")
            nc.scalar.dma_start(bias_sb[:], bias_in[:])
            epsb = constp.tile([128, 1], f32, name="epsb")
            nc.vector.memset(epsb, PD_EPS)

            # ---- W^T / t_pool: wt[j][p, o] = W[o, 16p+j] / t ----
            wl = []
            for l in range(oc):
                wli = wlp.tile([128, c_in], f32, name=f"wl{l}")
                dma_eng = nc.sync if l % 2 == 0 else nc.scalar
                dma_eng.dma_start(wli[:], w_in[l * 128:(l + 1) * 128, :])
                wl.append(wli)
            wt = []
            for j in range(cpp):
                pswt = psp.tile([128, o_out], f32, name="pswt", tag="rot")
                for l in range(oc):
                    src = wl[l].rearrange("o (p j) -> o p j", j=cpp)[:, :, j]
                    nc.tensor.transpose(pswt[:, l * 128:(l + 1) * 128],
                                        src, ident[:])
                wtj = wtp.tile([128, o_out], f32, name=f"wt{j}")
                nc.scalar.mul(wtj[:], pswt[:], 1.0 / t_pool)
                wt.append(wtj)

            # ---- persistent post tiles ----
            gl = [pp.tile([128, bc], f32, name=f"gl{m}") for m in range(oc)]
            gt = [pp.tile([128, b_full], f32, name=f"gt{m}") for m in range(oc)]
            rjrow = pp.tile([1, b_full], f32, name="rjrow")
            dlrow = pp.tile([1, bc], f32, name="dlrow")
            ri = pp.tile([bc, 1], f32, name="ri")
            outsb = pp.tile([bc, b_full], f32, name="outsb")
            gf = []

            idma = 0
            for ch in range(nch):
                # ---- pooling: one contiguous 1.6MB DMA per row ----
                p4c = lp.tile([128, cr, cpp], f32, name=f"p4_{ch}")
                for r in range(cr):
                    row = ch * cr + r
                    ft = fp.tile([128, 1, cpp, t_pool], f32, name="ft")
                    src = feat[row:row + 1, :, :].rearrange(
                        "b (p j) t -> p b j t", j=cpp)
                    dma_eng = nc.sync if idma % 2 == 0 else nc.scalar
                    dma_eng.dma_start(ft[:], src)
                    nc.vector.reduce_sum(p4c[:, r:r + 1, :], ft[:], axis=AX.X)
                    idma += 1

                # ---- projection chunk: [cr, o_out] ----
                gps = psg.tile([cr, o_out], f32, name="gps", tag="gps")
                for j in range(cpp):
                    nc.tensor.matmul(gps[:], p4c[:, :, j], wt[j][:],
                                     start=(j == 0), stop=False)
                nc.tensor.matmul(gps[:], ones1[:, :cr], bias_sb[:],
                                 start=False, stop=True)

                # ---- normalize rows (ACT square+accum, sqrt; DVE max/recip) ----
                gsb = sp.tile([cr, o_out], f32, name="gsb", tag="gsb")
                nc.scalar.copy(gsb[:], gps[:])
                scr = sp.tile([cr, o_out], f32, name="scr", tag="scr")
                nrm2 = sp.tile([cr, 1], f32, name="nrm2", tag="nrm2")
                nc.scalar.activation(scr[:], gsb[:], AF.Square,
                                     accum_out=nrm2[:])
                nrm = sp.tile([cr, 1], f32, name="nrm", tag="nrm")
                nc.scalar.sqrt(nrm[:], nrm2[:])
                nmax = sp.tile([cr, 1], f32, name="nmax", tag="nmax")
                nc.vector.tensor_scalar_max(nmax[:], nrm[:], NORM_EPS)
                rinv = sp.tile([cr, 1], f32, name="rinv", tag="rinv")
                nc.vector.reciprocal(rinv[:], nmax[:])
                gnc = sp.tile([cr, o_out], f32, name="gnc", tag="gnc")
                nc.scalar.mul(gnc[:], gsb[:], rinv[:])

                # local dist chunk: dl = ||gn + eps||, ri = 1/dl
                dl2 = sp.tile([cr, 1], f32, name="dl2", tag="dl2")
                nc.scalar.activation(scr[:], gnc[:], AF.Square,
                                     bias=epsb[:cr, :], accum_out=dl2[:])
                dlc = sp.tile([cr, 1], f32, name="dlc", tag="dlc")
                nc.scalar.sqrt(dlc[:], dl2[:])
                psdl = pst.tile([128, 128], f32, name="psdl", tag="tr")
                nc.tensor.transpose(psdl[:1, :cr], dlc[:], ident[:cr, :cr])
                nc.vector.tensor_copy(dlrow[:, ch * cr:(ch + 1) * cr],
                                      psdl[:1, :cr])

                # gl slices: [128 o-block, cr] transposes of local gn
                for m in range(oc):
                    psgl = pst.tile([128, 128], f32, name="psgl", tag="tr")
                    nc.tensor.transpose(psgl[:, :cr],
                                        gnc[:, m * 128:(m + 1) * 128],
                                        ident[:cr, :cr])
                    nc.vector.tensor_copy(gl[m][:, ch * cr:(ch + 1) * cr],
                                          psgl[:, :cr])

                # ---- AllGather this chunk's normalized rows ----
                ag_in = dp.tile([cr, o_out], f32, name=f"ag_in{ch}")
                ag_out = dp.tile([128, o_out], f32, name=f"ag_out{ch}",
                                 addr_space="Shared")
                nc.sync.dma_start(ag_in[:], gnc[:])
                nc.gpsimd.collective_compute(
                    "AllGather", AL.bypass,
                    replica_groups=[list(range(n_cores))],
                    ins=[ag_in.opt()], outs=[ag_out.opt()],
                )
                gfc = pp.tile([128, o_out], f32, name=f"gf{ch}")
                nc.sync.dma_start(gfc[:], ag_out[:])
                gf.append(gfc)

            # ---- gathered-side: dist + transposes + sim + divide ----
            scrq = sp.tile([128, o_out], f32, name="scrq", tag="scrq")
            for ch in range(nch):
                gfc = gf[ch]
                d2q = sp.tile([128, 1], f32, name="d2q", tag="d2q")
                nc.scalar.activation(scrq[:], gfc[:], AF.Square,
                                     bias=epsb[:], accum_out=d2q[:])
                dq = sp.tile([128, 1], f32, name="dq", tag="dq")
                nc.scalar.sqrt(dq[:], d2q[:])
                rjq = sp.tile([128, 1], f32, name="rjq", tag="rjq")
                nc.vector.reciprocal(rjq[:], dq[:])
                psrj = pst.tile([128, 128], f32, name="psrj", tag="tr")
                nc.tensor.transpose(psrj[:1, :], rjq[:], ident[:])
                nc.vector.tensor_copy(rjrow[:, ch * 128:(ch + 1) * 128],
                                      psrj[:1, :])
                for m in range(oc):
                    psgt = pst.tile([128, 128], f32, name="psgt", tag="tr")
                    nc.tensor.transpose(psgt[:],
                                        gfc[:, m * 128:(m + 1) * 128],
                                        ident[:])
                    nc.vector.tensor_copy(gt[m][:, ch * 128:(ch + 1) * 128],
                                          psgt[:])

            # local 1/dist column: transpose dlrow back to [bc, 1]
            psri = pst.tile([128, 128], f32, name="psri", tag="tr")
            nc.tensor.transpose(psri[:bc, :1], dlrow[:], ident[:1, :1])
            nc.vector.reciprocal(ri[:], psri[:bc, :1])

            for ch in range(nch):
                # sim block: [bc, 128] = gn_local @ gf[ch].T
                sps = psm.tile([bc, 128], f32, name="sps", tag="sim")
                for m in range(oc):
                    nc.tensor.matmul(sps[:], gl[m][:],
                                     gt[m][:, ch * 128:(ch + 1) * 128],
                                     start=(m == 0), stop=(m == oc - 1))
                # rden = min(1/dist_i, 1/dist_j, 1/eps) == 1/max(di, dj, eps)
                dps = psm.tile([bc, 128], f32, name="dps", tag="den")
                nc.tensor.matmul(dps[:], ones1[:],
                                 rjrow[:, ch * 128:(ch + 1) * 128],
                                 start=True, stop=True)
                rden = sp.tile([bc, 128], f32, name="rden", tag="rden")
                nc.vector.tensor_scalar(rden[:], dps[:], ri[:], 1.0 / DENO_EPS,
                                        op0=AL.min, op1=AL.min)
                # out columns for chunk ch: col = 64*rank + 16*ch + i
                ov = outsb.rearrange("b (r c i) -> b r c i",
                                     r=n_cores, i=cr)[:, :, ch, :]
                sv = sps.rearrange("b (r i) -> b r i", i=cr)
                rv = rden.rearrange("b (r i) -> b r i", i=cr)
                nc.vector.tensor_tensor(ov, sv, rv, op=AL.mult)

            nc.sync.dma_start(out_d[:], outsb[:])

    nc.compile()
    return nc


_NC_CACHE = {}


def _get_nc():
    key = (B_FULL, C_IN, T_POOL, O_OUT, N_CORES)
    if key not in _NC_CACHE:
        _NC_CACHE[key] = build_kernel(*key)
    return _NC_CACHE[key]


def _run(features, W, bias, trace=False):
    from concourse.bass_utils import run_bass_kernel_spmd

    feats = np.ascontiguousarray(np.asarray(features, dtype=np.float32))
    w_np = np.ascontiguousarray(np.asarray(W, dtype=np.float32))
    bias_np = np.ascontiguousarray(
        np.asarray(bias, dtype=np.float32).reshape(1, O_OUT))
    bc = B_FULL // N_CORES

    nc = _get_nc()
    in_maps = [
        {"features": feats[r * bc:(r + 1) * bc], "w": w_np, "bias": bias_np}
        for r in range(N_CORES)
    ]
    res = run_bass_kernel_spmd(nc, in_maps, core_ids=list(range(N_CORES)),
                               trace=trace)
    out = np.concatenate([res.results[r]["out"] for r in range(N_CORES)], axis=0)
    return out, res.exec_time_ns


def kernel(features, W, bias):
    out, _ = _run(features, W, bias)
    return out
